# revision 8
# baseline (speedup 1.0000x reference)
"""Two-layer GCN (GCNConv 256->128->64, leaky_relu) on 8 Trainium2 NeuronCores.

v2: full-bf16 feature path, self-loops folded into the edge stream, fused
leaky-relu on the scalar engine, batched dense-layer DMAs.

Strategy (graph/data parallel):
  - Nodes sharded contiguously across 8 cores (12500 each); each core owns
    the destination rows of the scatter.
  - Symmetric norm folded into features: x_pre = dinv*x (host), so
    h~ = x_pre @ W1 and out[v] = dinv[v]*(sum_{e: dst=v} h~[src_e]) + b
    where the edge stream includes the self loop (v,v).
  - Per layer: dense transform -> AllGather h~ (bf16) -> per 128-node dst
    block, dma_gather source rows (256B/row) and segment-sum them with 0/1
    selection-matrix bf16 matmuls accumulated in PSUM.
  - h2 is stored padded to 128 features (cols 64..127 zero via a padded W2)
    so layer-2 gather rows are also 256B.
  - Edge tables (int16 gather indices into 32768-row windows, bf16 dst
    slots) are precomputed host-side; edges sorted by (dst-block group,
    src window, dst block), padded to 128-edge tiles shared across cores.
"""
import math
import sys

import numpy as np

sys.path.insert(0, "/opt/trn_rl_repo")

import concourse.bacc as bacc  # noqa: E402
import concourse.mybir as mybir  # noqa: E402
import concourse.tile as tile  # noqa: E402
from concourse.bass_utils import run_bass_kernel_spmd  # noqa: E402

P = 128
F32 = mybir.dt.float32
BF16 = mybir.dt.bfloat16
I16 = mybir.dt.int16
NPBF16 = mybir.dt.np(BF16)


class Cfg:
    def __init__(self, n, e, d0, d1, d2, ncores=8, grp=6, chunk=32768,
                 neg_slope=0.01, nq=4):
        assert n % ncores == 0
        self.nq = nq
        self.n, self.e = n, e
        self.d0, self.d1, self.d2 = d0, d1, d2
        self.ncores = ncores
        self.nloc = n // ncores
        self.nblk = math.ceil(self.nloc / P)
        self.grp = grp
        self.ngrp = math.ceil(self.nblk / grp)
        self.chunk = chunk
        self.nchunk = math.ceil(n / chunk)
        self.neg_slope = neg_slope

    def rows(self, b):
        return min(P, self.nloc - b * P)


FULL = Cfg(n=100000, e=1600000, d0=256, d1=128, d2=64, grp=2)


# --------------------------------------------------------------------------
# host-side preprocessing
# --------------------------------------------------------------------------

def prepare(cfg, x, edge_index, W1, b1, W2, b2):
    src0 = np.asarray(edge_index[0], dtype=np.int64)
    dst0 = np.asarray(edge_index[1], dtype=np.int64)
    deg = (np.bincount(dst0, minlength=cfg.n) + 1).astype(np.float32)
    dinv = (np.float32(1.0) / np.sqrt(deg)).astype(np.float32)

    # self loops folded into the edge stream
    loops = np.arange(cfg.n, dtype=np.int64)
    src = np.concatenate([src0, loops])
    dst = np.concatenate([dst0, loops])

    core = dst // cfg.nloc
    dstl = dst - core * cfg.nloc
    b = dstl >> 7
    k = src // cfg.chunk
    g = b // cfg.grp
    key = ((core * cfg.ngrp + g) * cfg.nchunk + k) * cfg.nblk + b
    order = np.argsort(key, kind="stable")
    src_s, b_s, k_s, core_s, dstl_s = (
        src[order], b[order], k[order], core[order], dstl[order])

    seg_id = ((core_s * cfg.nblk + b_s) * cfg.nchunk + k_s)
    counts = np.bincount(seg_id, minlength=cfg.ncores * cfg.nblk * cfg.nchunk)
    counts = counts.reshape(cfg.ncores, cfg.nblk, cfg.nchunk)
    T = np.ceil(counts.max(axis=0) / P).astype(np.int64)  # [nblk, nchunk]

    seg_off = np.zeros((cfg.nblk, cfg.nchunk), np.int64)
    blocks_of_g = [list(range(gg * cfg.grp, min((gg + 1) * cfg.grp, cfg.nblk)))
                   for gg in range(cfg.ngrp)]
    tot_tiles = 0
    for gg in range(cfg.ngrp):
        for kk in range(cfg.nchunk):
            for bb in blocks_of_g[gg]:
                seg_off[bb, kk] = tot_tiles * P
                tot_tiles += int(T[bb, kk])
    etot = tot_tiles * P

    xs = np.asarray(x, dtype=np.float32)
    W1b = np.asarray(W1, np.float32).astype(NPBF16)
    W2p = np.zeros((cfg.d1, P), np.float32)
    W2p[:, :cfg.d2] = np.asarray(W2, np.float32)
    W2p = W2p.astype(NPBF16)
    b1bc = np.tile(np.asarray(b1, np.float32)[None, :], (P, 1)).astype(NPBF16)
    b2bc = np.tile(np.asarray(b2, np.float32)[None, :], (P, 1)).astype(
        np.float32)

    in_maps = []
    for c in range(cfg.ncores):
        m = core_s == c
        src_c, b_c, k_c, dstl_c = src_s[m], b_s[m], k_s[m], dstl_s[m]
        seg_c = (b_c * cfg.nchunk + k_c)
        ne = len(src_c)
        if ne:
            newseg = np.r_[True, seg_c[1:] != seg_c[:-1]]
            seg_start = np.maximum.accumulate(
                np.where(newseg, np.arange(ne), 0))
            pos_in_seg = np.arange(ne) - seg_start
        else:
            pos_in_seg = np.zeros(0, np.int64)
        padded_pos = seg_off[b_c, k_c] + pos_in_seg

        seg_calls = getattr(cfg, "seg_calls", True)
        fill = -1 if seg_calls else 0
        idx_pad = np.full(etot, fill, np.int16)
        idx_pad[padded_pos] = (src_c - k_c * cfg.chunk).astype(np.int16)
        dr_pad = np.full(etot, -1.0, np.float32)
        dr_pad[padded_pos] = (dstl_c - b_c * P).astype(np.float32)

        idx16 = idx_pad.reshape(etot // 16, 16).T.copy()
        idx_tab = np.tile(idx16, (8, 1))
        drel_tab = dr_pad.reshape(tot_tiles, P).T.copy()
        seg_cnt = counts[c].reshape(1, cfg.nblk * cfg.nchunk).astype(np.int32)

        lo = c * cfg.nloc
        # x_pre = dinv * x, bf16, wrapped: xw[p, k0*nloc + col] =
        #   x_pre[col, k0*128+p]
        xp = (xs[lo:lo + cfg.nloc] * dinv[lo:lo + cfg.nloc, None]).astype(
            NPBF16)
        xT = np.ascontiguousarray(xp.T)                      # [256, nloc]
        xw = xT.reshape(2, P, cfg.nloc).transpose(1, 0, 2).reshape(
            P, 2 * cfg.nloc).copy()
        dloc = dinv[lo:lo + cfg.nloc]
        dcols = np.ones(cfg.nblk * P, np.float32)
        dcols[:cfg.nloc] = dloc
        dinv_cols = dcols.reshape(cfg.nblk, P).T.copy()

        in_maps.append({
            "xw": xw, "dinv_cols": dinv_cols,
            "W1": W1b, "W2p": W2p, "b1bc": b1bc, "b2bc": b2bc,
            "idx_tab": idx_tab, "drel_tab": drel_tab, "seg_cnt": seg_cnt,
        })

    struct = dict(T=T, blocks_of_g=blocks_of_g, tot_tiles=tot_tiles)
    return in_maps, struct


# --------------------------------------------------------------------------
# device program
# --------------------------------------------------------------------------

def build_program(cfg, struct, repeat=1):
    T = struct["T"]
    blocks_of_g = struct["blocks_of_g"]
    tot_tiles = struct["tot_tiles"]
    NB, NK, NG = cfg.nblk, cfg.nchunk, cfg.ngrp
    D0, D1, D2 = cfg.d0, cfg.d1, cfg.d2
    K0 = D0 // P
    G = cfg.grp

    grp_tiles = []           # tiles per group
    gk_info = []             # [g][k] -> (col0 within group, ntiles)
    blk_tile_cols = {}       # b -> [(col within group, ntiles, k)]
    grp_col0 = []            # group offset in global tile stream
    tot = 0
    for g in range(NG):
        grp_col0.append(tot)
        col = 0
        ks = []
        for k in range(NK):
            c0 = col
            for b in blocks_of_g[g]:
                t = int(T[b, k])
                if t:
                    blk_tile_cols.setdefault(b, []).append((col, t, k))
                    col += t
            ks.append((c0, col - c0))
        gk_info.append(ks)
        grp_tiles.append(col)
        tot += col
    assert tot == tot_tiles
    TG = max(grp_tiles)

    nc = bacc.Bacc("TRN2", target_bir_lowering=False, debug=False,
                   num_devices=cfg.ncores, num_swdge_queues=cfg.nq)
    xw_t = nc.dram_tensor("xw", [P, K0 * cfg.nloc], BF16,
                          kind="ExternalInput")
    dinv_t = nc.dram_tensor("dinv_cols", [P, NB], F32, kind="ExternalInput")
    W1 = nc.dram_tensor("W1", [D0, D1], BF16, kind="ExternalInput")
    W2p = nc.dram_tensor("W2p", [D1, P], BF16, kind="ExternalInput")
    b1t = nc.dram_tensor("b1bc", [P, D1], BF16, kind="ExternalInput")
    b2t = nc.dram_tensor("b2bc", [P, D2], F32, kind="ExternalInput")
    idx_t = nc.dram_tensor("idx_tab", [P, tot_tiles * 8], I16,
                           kind="ExternalInput")
    drel_t = nc.dram_tensor("drel_tab", [P, tot_tiles], F32,
                            kind="ExternalInput")
    cnt_t = nc.dram_tensor("seg_cnt", [1, NB * NK], mybir.dt.int32,
                           kind="ExternalInput")
    out_t = nc.dram_tensor("out_loc", [cfg.nloc, D2], F32,
                           kind="ExternalOutput")
    seg_calls = getattr(cfg, "seg_calls", True)

    eq = mybir.AluOpType.is_equal
    mul = mybir.AluOpType.mult
    add = mybir.AluOpType.add
    lrelu = mybir.ActivationFunctionType.Lrelu

    with tile.TileContext(nc) as tc:
        with (
            tc.tile_pool(name="const", bufs=1) as cp,
            tc.tile_pool(name="work", bufs=2) as wp,
            tc.tile_pool(name="sm", bufs=5) as smp,
            tc.tile_pool(name="mm", bufs=4, space="PSUM") as mmp,
            tc.tile_pool(name="agg", bufs=3, space="PSUM") as aggp,
            tc.tile_pool(name="dram", bufs=1, space="DRAM") as drp,
        ):
            h1_loc = drp.tile([cfg.nloc, D1], BF16, tag="h1_loc")
            h2_loc = drp.tile([cfg.nloc, P], BF16, tag="h2_loc")

            w1sb = cp.tile([P, K0, D1], BF16)
            for k0 in range(K0):
                nc.sync.dma_start(out=w1sb[:, k0, :],
                                  in_=W1[k0 * P:(k0 + 1) * P, :])
            w2sb = cp.tile([P, P], BF16)
            nc.sync.dma_start(out=w2sb[:], in_=W2p[:])
            b1sb = cp.tile([P, D1], BF16)
            nc.sync.dma_start(out=b1sb[:], in_=b1t[:])
            b2sb = cp.tile([P, D2], F32)
            nc.sync.dma_start(out=b2sb[:], in_=b2t[:])
            dvsb = cp.tile([P, NB], F32)
            nc.sync.dma_start(out=dvsb[:], in_=dinv_t[:])
            iota_f = cp.tile([P, P], F32)
            nc.gpsimd.iota(iota_f[:], pattern=[[1, P]], base=0,
                           channel_multiplier=0,
                           allow_small_or_imprecise_dtypes=True)
            iotab = cp.tile([P, P], BF16)
            nc.vector.tensor_copy(out=iotab[:], in_=iota_f[:])
            identb = cp.tile([P, P], BF16)
            from concourse.masks import make_identity
            make_identity(nc, identb[:])
            cnt_sb = cp.tile([1, NB * NK], mybir.dt.int32)
            nc.sync.dma_start(out=cnt_sb[:], in_=cnt_t[:])
            ni_reg = nc.gpsimd.alloc_register(name="ni_reg") \
                if seg_calls else None

            # ---------------- dense layer 1: h1_loc = x_pre @ W1 (bf16)
            def dense1():
                ST = 8
                nsup = math.ceil(NB / ST)
                for s in range(nsup):
                    blo = s * ST
                    bhi = min(blo + ST, NB)
                    c0 = blo * P
                    w = min(bhi * P, cfg.nloc) - c0
                    xt = wp.tile([P, K0, ST * P], BF16, tag="xt")
                    xin = xw_t[:].rearrange("p (k c) -> p k c", k=K0)
                    nc.sync.dma_start(out=xt[:, :, :w],
                                      in_=xin[:, :, c0:c0 + w])
                    full = (w == ST * P)
                    h1w = wp.tile([P, ST, D1], BF16, tag="h1w")
                    for j, bb in enumerate(range(blo, bhi)):
                        nb = cfg.rows(bb)
                        ps = mmp.tile([P, D1], F32, tag="mm")
                        for k0 in range(K0):
                            nc.tensor.matmul(
                                out=ps[:nb, :],
                                lhsT=xt[:, k0, j * P:j * P + nb],
                                rhs=w1sb[:, k0, :],
                                start=(k0 == 0), stop=(k0 == K0 - 1))
                        nc.scalar.copy(out=h1w[:nb, j, :], in_=ps[:nb, :])
                        if not full:
                            nc.sync.dma_start(
                                out=h1_loc[bb * P:bb * P + nb, :],
                                in_=h1w[:nb, j, :])
                    if full:
                        oview = h1_loc[c0:c0 + ST * P, :].rearrange(
                            "(j p) f -> p j f", p=P)
                        nc.sync.dma_start(out=oview, in_=h1w[:, :, :])

            # ---------------- aggregation (both layers)
            def agg_phase(layer, h_full, D_out, sink,
                          gather_only=False, compute_only=False):
                sel_batch = getattr(cfg, "sel_batch", True)
                for g in range(NG):
                    Tg = grp_tiles[g]
                    if Tg == 0:
                        continue
                    gt0 = grp_col0[g]
                    stage_raw = wp.tile([P, TG * D1], BF16, tag="stage",
                                        bufs=6)
                    stage = stage_raw[:].rearrange("p (t d) -> p t d", d=D1)
                    idxt = wp.tile([P, TG * 8], I16, tag="idx", bufs=4)
                    drt = wp.tile([P, TG], F32, tag="drel", bufs=4)
                    nc.sync.dma_start(out=idxt[:, :Tg * 8],
                                      in_=idx_t[:, gt0 * 8:(gt0 + Tg) * 8])
                    nc.sync.dma_start(out=drt[:, :Tg],
                                      in_=drel_t[:, gt0:gt0 + Tg])
                    if compute_only:
                        nc.vector.memset(stage_raw[:], 0.0)
                    elif seg_calls:
                        # one gather per (b,k) segment: trailing -1 pads are
                        # skipped by the DMA; runtime count in ni_reg.
                        if g < 6:
                            nc.vector.memset(stage_raw[:], 0.0)
                        qrr = 0
                        for b in blocks_of_g[g]:
                            for (col, t, k) in blk_tile_cols.get(b, []):
                                hi = min((k + 1) * cfg.chunk, cfg.n)
                                j = b * NK + k
                                nc.gpsimd.reg_load(ni_reg,
                                                   cnt_sb[0:1, j:j + 1])
                                nc.gpsimd.dma_gather(
                                    out_ap=stage[:, col:col + t, :],
                                    in_ap=h_full[k * cfg.chunk:hi, :],
                                    idxs_ap=idxt[:, col * 8:(col + t) * 8],
                                    num_idxs=t * P, num_idxs_reg=ni_reg,
                                    elem_size=D1, single_packet=False,
                                    queue_num=qrr % cfg.nq)
                                qrr += 1
                    else:
                        qrr = 0
                        for k in range(NK):
                            c0, ntk = gk_info[g][k]
                            if ntk == 0:
                                continue
                            hi = min((k + 1) * cfg.chunk, cfg.n)
                            nc.gpsimd.dma_gather(
                                out_ap=stage[:, c0:c0 + ntk, :],
                                in_ap=h_full[k * cfg.chunk:hi, :],
                                idxs_ap=idxt[:, c0 * 8:(c0 + ntk) * 8],
                                num_idxs=ntk * P, num_idxs_reg=ntk * P,
                                elem_size=D1, single_packet=False,
                                queue_num=qrr % cfg.nq)
                            qrr += 1
                    if gather_only:
                        gacc = wp.tile([P, D1], F32, tag="gacc", bufs=3)
                        nc.vector.tensor_copy(out=gacc[:], in_=stage[:, 0, :])
                        continue
                    smgs = {}
                    if sel_batch:
                        # one is_equal per (g,k) window:
                        # sel[p, t, c] = (drt[p, t] == iota[c])
                        for k in range(NK):
                            c0, ntk = gk_info[g][k]
                            if ntk == 0:
                                continue
                            smg_raw = smp.tile([P, ntk * P], BF16, tag="smg")
                            smg = smg_raw[:].rearrange(
                                "p (t c) -> p t c", c=P)
                            nc.vector.tensor_tensor(
                                out=smg[:, :, :],
                                in0=drt[:, c0:c0 + ntk].unsqueeze(2)
                                .broadcast_to([P, ntk, P]),
                                in1=iota_f[:].unsqueeze(1).broadcast_to(
                                    [P, ntk, P]),
                                op=eq)
                            smgs[k] = smg
                    agg = aggp.tile([P, G * D1], F32, tag="agg")
                    for slot, b in enumerate(blocks_of_g[g]):
                        segs = blk_tile_cols.get(b, [])
                        ntot = sum(t for _, t, _ in segs)
                        nb = cfg.rows(b)
                        assert ntot > 0
                        left = ntot
                        first = True
                        for (col, t, k) in segs:
                            for tt in range(t):
                                if sel_batch:
                                    kc0 = gk_info[g][k][0]
                                    sm = smgs[k][:, col - kc0 + tt, :]
                                else:
                                    smt = smp.tile([P, P], BF16, tag="sm")
                                    nc.vector.tensor_scalar(
                                        out=smt[:], in0=iotab[:],
                                        scalar1=drt[:, col + tt:col + tt + 1],
                                        scalar2=None, op0=eq)
                                    sm = smt[:]
                                nc.tensor.matmul(
                                    out=agg[:, slot * D1:slot * D1 + D_out],
                                    lhsT=sm,
                                    rhs=stage[:, col + tt, :D_out],
                                    start=first, stop=(left == 1),
                                    skip_group_check=True)
                                first = False
                                left -= 1
                        sink(g, slot, b, nb, agg)

            def l1_sink(g, slot, b, nb, agg):
                t2 = wp.tile([P, D1], BF16, tag="t2", bufs=3)
                nc.vector.tensor_scalar(
                    out=t2[:nb, :], in0=agg[:nb, slot * D1:(slot + 1) * D1],
                    scalar1=dvsb[:nb, b:b + 1], scalar2=None, op0=mul)
                t2b = wp.tile([P, D1], BF16, tag="t2b", bufs=3)
                nc.vector.tensor_tensor(out=t2b[:nb, :], in0=t2[:nb, :],
                                        in1=b1sb[:nb, :], op=add)
                # t3 = lrelu(dinv * t2b) = dinv * lrelu(t2b)  (dinv > 0)
                t3 = wp.tile([P, D1], BF16, tag="t3", bufs=3)
                nc.scalar.activation(out=t3[:nb, :], in_=t2b[:nb, :],
                                     func=lrelu, bias=0.0,
                                     scale=dvsb[:nb, b:b + 1],
                                     alpha=float(cfg.neg_slope))
                tp = mmp.tile([P, P], BF16, tag="mm")
                nc.tensor.transpose(out=tp[:, :nb], in_=t3[:nb, :],
                                    identity=identb[:nb, :nb])
                t4 = wp.tile([P, P], BF16, tag="t4", bufs=3)
                nc.scalar.copy(out=t4[:, :nb], in_=tp[:, :nb])
                v = mmp.tile([P, P], F32, tag="mm")
                nc.tensor.matmul(out=v[:nb, :], lhsT=t4[:, :nb],
                                 rhs=w2sb[:], start=True, stop=True)
                h2b = wp.tile([P, P], BF16, tag="h2b", bufs=3)
                nc.scalar.copy(out=h2b[:nb, :], in_=v[:nb, :])
                nc.sync.dma_start(out=h2_loc[b * P:b * P + nb, :],
                                  in_=h2b[:nb, :])

            def l2_sink(g, slot, b, nb, agg):
                t2 = wp.tile([P, D2], F32, tag="u2", bufs=3)
                nc.vector.tensor_scalar(
                    out=t2[:nb, :], in0=agg[:nb, slot * D1:slot * D1 + D2],
                    scalar1=dvsb[:nb, b:b + 1], scalar2=None, op0=mul)
                uo = wp.tile([P, D2], F32, tag="uo", bufs=3)
                nc.vector.tensor_tensor(out=uo[:nb, :], in0=t2[:nb, :],
                                        in1=b2sb[:nb, :], op=add)
                nc.sync.dma_start(out=out_t[b * P:b * P + nb, :],
                                  in_=uo[:nb, :])

            mode = getattr(cfg, "repeat_mode", "all")
            h1_full = h2_full = None
            for _rep in range(repeat):
                rep_all = mode == "all" or _rep == 0
                if rep_all or mode == "collectives":
                    h1_full = drp.tile([cfg.n, D1], BF16,
                                       tag=f"h1_full{_rep}",
                                       addr_space="Shared",
                                       name=f"h1_full{_rep}")
                    h2_full = drp.tile([cfg.n, P], BF16,
                                       tag=f"h2_full{_rep}",
                                       addr_space="Shared",
                                       name=f"h2_full{_rep}")
                if rep_all or mode == "dense":
                    dense1()
                if rep_all or mode == "collectives":
                    nc.gpsimd.collective_compute(
                        "AllGather", mybir.AluOpType.bypass,
                        replica_groups=[list(range(cfg.ncores))],
                        ins=[h1_loc.opt()], outs=[h1_full.opt()])
                if rep_all or mode in ("agg", "agg1"):
                    agg_phase(1, h1_full, D1, l1_sink)
                elif mode == "gather1":
                    agg_phase(1, h1_full, D1, l1_sink, gather_only=True)
                elif mode == "compute1":
                    agg_phase(1, h1_full, D1, l1_sink, compute_only=True)
                if rep_all or mode == "collectives":
                    nc.gpsimd.collective_compute(
                        "AllGather", mybir.AluOpType.bypass,
                        replica_groups=[list(range(cfg.ncores))],
                        ins=[h2_loc.opt()], outs=[h2_full.opt()])
                if rep_all or mode in ("agg", "agg2"):
                    agg_phase(2, h2_full, D2, l2_sink)
                elif mode == "gather2":
                    agg_phase(2, h2_full, D2, l2_sink, gather_only=True)
                elif mode == "compute2":
                    agg_phase(2, h2_full, D2, l2_sink, compute_only=True)

    nc.compile()
    return nc


# --------------------------------------------------------------------------
# entry point
# --------------------------------------------------------------------------

_CACHE = {}


def _run(cfg, inputs):
    in_maps, struct = prepare(cfg, inputs["x"], inputs["edge_index"],
                              inputs["W1"], inputs["b1"],
                              inputs["W2"], inputs["b2"])
    key = (cfg.n, cfg.e, struct["T"].tobytes())
    nc = _CACHE.get(key)
    if nc is None:
        nc = build_program(cfg, struct)
        _CACHE[key] = nc
    res = run_bass_kernel_spmd(nc, in_maps, list(range(cfg.ncores)))
    out = np.concatenate([res.results[c]["out_loc"]
                          for c in range(cfg.ncores)], axis=0)
    return out.astype(np.float32)


def kernel(x, edge_index, W1, b1, W2, b2):
    return _run(FULL, dict(x=x, edge_index=edge_index, W1=W1, b1=b1,
                           W2=W2, b2=b2))
